# revision 2
# baseline (speedup 1.0000x reference)
"""Trainium2 Bass kernel for nn_BitModel (MLGRU step + BitGLU, ternary weights).

Data-parallel over batch (512 rows/core, no collectives). Ternary weights
are exact in every dtype used. Stream-precision scheme (chosen from a
calibrated numpy error study; HW fp32r == round-to-nearest-12-bit-mantissa,
fp16 == 11 bits, both at ~1 matmul slot per 128-K; fp32r slots are +5%):

  x  -> f/c/g gates : fp32r single stream (12-bit)   [was fp16+fp8lo, 1.5x]
  gh -> out_proj    : fp32r
  o  -> proj_u/proj_g : fp16 (1.45e-2 total, under the 2e-2 gate; keeps
                        phase 3 all-f16 at 216ns slots with no dtype switches)
  gu -> proj_out    : fp16 (pre-scaled 1/16 to fit range)

All activations on ScalarE are Sigmoid (silu computed as (t+b)*sigmoid on
VectorE via scalar_tensor_tensor) so the ACT table loads once, removing
~16us of TensorE stalls the baseline had from per-chunk table reloads.
"""

import sys

sys.path.insert(0, "/opt/trn_rl_repo")

import numpy as np

import concourse.bass as bass
import concourse.mybir as mybir
import concourse.tile as tile
from concourse.vector_clock import ScopedClock

DIM = 2048
HID = 8192
BATCH = 4096
NCORES = 8
B = BATCH // NCORES
P = 128
JC_D = DIM // P   # 16
JC_H = HID // P   # 64
THRESH = 0.33
GU_SCALE = 16.0

F16 = mybir.dt.float16
F32 = mybir.dt.float32
F32R = mybir.dt.float32r

# stream dtype config (fallbacks: flip to F16 if f32r DVE-writes fail)
GH_DT = F32R
OG_DT = F16

# bias column layout in the packed [128, 208] bias tensor
COL_NF = 0
COL_C = 16
COL_G = 32
COL_O = 48
COL_U = 64
COL_G2 = 128
COL_Y = 192
N_BIAS_COLS = 208

AF = mybir.ActivationFunctionType
ALU = mybir.AluOpType


def _patch_tile_drain():
    """This walrus build rejects instructions carrying >~2 attached sem
    waits; re-emit Tile's kernel-tail drain waits as standalone wait_ge."""
    if getattr(tile.TileContext, "_drain_patched", False):
        return

    def _drain_and_barrier(self, tick_clock, wait_clock):
        nc = self.nc
        probe = nc.sync.nop(nofuse=True)
        wait_clock.add_sem_waits(
            probe.ins, ScopedClock({None: tick_clock.global_clock})
        )
        si = probe.ins.sync_info
        waits = list(si.on_wait) if si else []
        if si:
            si.on_wait = []
        handles = {h.name: h for h in self.sems.allocated().values()}
        for w in waits:
            nc.sync.wait_ge(handles[w.ant_name], w.wait_value)
        nc.sync.drain()
        nc.all_engine_barrier()
        assert self.sems is not None
        popped = nc._tile_sem_poison_stack.pop()
        assert popped is self._sem_poison
        nc.clear_and_free_semaphores(list(self.sems.allocated().values()))
        nc.all_engine_barrier()

    tile.TileContext._drain_and_barrier = _drain_and_barrier
    tile.TileContext._drain_patched = True


_patch_tile_drain()


def _split_excess_waits(nc, cap=1):
    """Rewrite every instruction with >cap attached sem waits into a chain
    of single-wait InstEventSemaphore ops + the instruction with cap waits."""
    ctr = 0
    for f in nc.m.functions:
        for bb in f.blocks:
            il = bb.instructions
            i = 0
            while i < len(il):
                inst = il[i]
                si = inst.sync_info
                waits = list(si.on_wait) if si else []
                if len(waits) > cap:
                    extra, keep = waits[:-cap], waits[-cap:]
                    evs = []
                    for w in extra:
                        ev = mybir.InstEventSemaphore(
                            name=f"waitsplit-{ctr}", ins=[], outs=[]
                        )
                        ctr += 1
                        ev.engine = inst.engine
                        ev.sync_info = mybir.SyncInfo(on_wait=[w], on_update=[])
                        evs.append(ev)
                    si.on_wait = keep
                    il[i:i] = evs
                    i += len(evs)
                i += 1
    return ctr


def _ternary(w):
    w = np.asarray(w, np.float32)
    return np.where(np.abs(w) < THRESH, 0.0, np.sign(w)).astype(np.float32)


def _pack_weight(w, dtype=np.float16):
    """[out_f, in_f] f32 -> ternarized, transposed, tiled [jc, p, ko, j]
    with element = tern(w)[jc*128+j, ko*128+p]."""
    of, inf_ = w.shape
    jc, ko = of // P, inf_ // P
    t = _ternary(w).reshape(jc, P, ko, P)          # [jc, j, ko, p]
    t = np.ascontiguousarray(t.transpose(0, 3, 2, 1))  # [jc, p, ko, j]
    return t.astype(dtype)


def _pack_bias_col(b):
    return np.ascontiguousarray(np.asarray(b, np.float32).reshape(-1, P).T)


def _build_nc():
    nc = bass.Bass()

    xr = nc.declare_dram_parameter("xr", [P, JC_D, B], F32R, isOutput=False)
    wfr = nc.declare_dram_parameter("wfr", [JC_D, P, JC_D, P], F32R, isOutput=False)
    wcr = nc.declare_dram_parameter("wcr", [JC_D, P, JC_D, P], F32R, isOutput=False)
    wgr = nc.declare_dram_parameter("wgr", [JC_D, P, JC_D, P], F32R, isOutput=False)
    wo = nc.declare_dram_parameter("wo", [JC_D, P, JC_D, P], GH_DT, isOutput=False)
    wu = nc.declare_dram_parameter("wu", [JC_H, P, JC_D, P], F16, isOutput=False)
    wg2 = nc.declare_dram_parameter("wg2", [JC_H, P, JC_D, P], OG_DT, isOutput=False)
    wo2 = nc.declare_dram_parameter(
        "wo2", [JC_D, 2, P, JC_H // 2, P], F16, isOutput=False
    )
    biases = nc.declare_dram_parameter("biases", [P, N_BIAS_COLS], F32, isOutput=False)
    out = nc.declare_dram_parameter("out", [JC_D, P, B], F32, isOutput=True)

    from contextlib import ExitStack

    with tile.TileContext(nc) as tc:
        with (
            tc.tile_pool(name="const", bufs=1) as const,
            tc.tile_pool(name="wpool", bufs=4) as wpool,
            tc.tile_pool(name="psum", bufs=8, space="PSUM") as psum,
        ):
            def bias_ap(col):
                return bias_sb[:, col : col + 1]

            es_gh = ExitStack()
            gh_pool = es_gh.enter_context(tc.tile_pool(name="gh_pool", bufs=1))
            gh_sb = gh_pool.tile([P, JC_D, B], GH_DT)
            es_o = ExitStack()

            # ---- phase 1: gates; gh = g * ((1-f)*c), all-sigmoid ACT ----
            with (
                tc.tile_pool(name="x_pool", bufs=1) as x_pool,
                tc.tile_pool(name="tmp1", bufs=2) as tmp,
            ):
                # x lands as 8 separate 2-chunk tiles so the first matmuls
                # depend only on their own piece (a sliced single tile makes
                # the first matmul wait for ALL x DMAs, ~12us of dead time);
                # first-jc c/g slabs interleave into the x stream
                wf0_sb = wpool.tile([P, JC_D, P], F32R, tag="wr1m")
                nc.sync.dma_start(out=wf0_sb[:], in_=wfr[0])
                XCH = 1
                wc0_sb = wpool.tile([P, JC_D, P], F32R, tag="wr1m")
                wg0_sb = wpool.tile([P, JC_D, P], F32R, tag="wr1m")
                x_pieces = []
                for kc in range(JC_D // XCH):
                    xp = x_pool.tile([P, XCH, B], F32R, name=f"xp{kc}")
                    nc.sync.dma_start(out=xp[:], in_=xr[:, kc * XCH:(kc + 1) * XCH])
                    x_pieces.append(xp)
                    if kc == 3:
                        nc.sync.dma_start(out=wc0_sb[:], in_=wcr[0])
                    if kc == 5:
                        nc.sync.dma_start(out=wg0_sb[:], in_=wgr[0])
                bias_sb = const.tile([P, N_BIAS_COLS], F32)
                nc.sync.dma_start(out=bias_sb[:], in_=biases[:])

                def xap(ko):
                    return x_pieces[ko // XCH][:, ko % XCH]

                for jc in range(JC_D):
                    if jc == 0:
                        wf_sb = wf0_sb
                    else:
                        wf_sb = wpool.tile([P, JC_D, P], F32R, tag="wr1m")
                        nc.sync.dma_start(out=wf_sb[:], in_=wfr[jc])
                    ps_f = psum.tile([P, B], F32, tag="ps")
                    for ko in range(JC_D):
                        nc.tensor.matmul(ps_f, wf_sb[:, ko], xap(ko),
                                         start=(ko == 0), stop=(ko == JC_D - 1))

                    if jc == 0:
                        wc_sb = wc0_sb
                    else:
                        wc_sb = wpool.tile([P, JC_D, P], F32R, tag="wr1m")
                        nc.sync.dma_start(out=wc_sb[:], in_=wcr[jc])
                    ps_c = psum.tile([P, B], F32, tag="ps")
                    for ko in range(JC_D):
                        nc.tensor.matmul(ps_c, wc_sb[:, ko], xap(ko),
                                         start=(ko == 0), stop=(ko == JC_D - 1))

                    if jc == 0:
                        wg_sb = wg0_sb
                    else:
                        wg_sb = wpool.tile([P, JC_D, P], F32R, tag="wr1m")
                        nc.sync.dma_start(out=wg_sb[:], in_=wgr[jc])
                    ps_g = psum.tile([P, B], F32, tag="ps")
                    for ko in range(JC_D):
                        nc.tensor.matmul(ps_g, wg_sb[:, ko], xap(ko),
                                         start=(ko == 0), stop=(ko == JC_D - 1))

                    # 1-f = sigmoid(-(t_f+b_f)); bias col holds -b_f
                    onemf = tmp.tile([P, B], F32, tag="onemf")
                    nc.scalar.activation(
                        onemf, ps_f, AF.Sigmoid, bias=bias_ap(COL_NF + jc), scale=-1.0
                    )
                    sc = tmp.tile([P, B], F32, tag="sc")
                    nc.scalar.activation(sc, ps_c, AF.Sigmoid, bias=bias_ap(COL_C + jc))
                    g_sb = tmp.tile([P, B], F32, tag="g")
                    nc.scalar.activation(g_sb, ps_g, AF.Sigmoid, bias=bias_ap(COL_G + jc))
                    # c = silu(t_c+b_c) = (t_c+b_c)*sigmoid(t_c+b_c)
                    c_sb = tmp.tile([P, B], F32, tag="c")
                    nc.vector.scalar_tensor_tensor(
                        c_sb, ps_c, bias_ap(COL_C + jc), sc, ALU.add, ALU.mult
                    )
                    h_sb = tmp.tile([P, B], F32, tag="h")
                    nc.vector.tensor_tensor(h_sb, onemf, c_sb, ALU.mult)
                    nc.vector.tensor_tensor(gh_sb[:, jc], g_sb, h_sb, ALU.mult)

            o_pool = es_o.enter_context(
                tc.tile_pool(name="o_pool", bufs=1, side="right")
            )
            o16_sb = o_pool.tile([P, JC_D, B], F16)
            og_sb = o16_sb if OG_DT == F16 else o_pool.tile([P, JC_D, B], OG_DT)

            # ---- phase 2: o = out_proj(gh) + b -> fp16 (u) + f32r (g2) ----
            with tc.tile_pool(name="tmp2", bufs=2) as tmp:
                for jc in range(JC_D):
                    wo_sb = wpool.tile([P, JC_D, P], GH_DT, tag="wr1m")
                    nc.sync.dma_start(out=wo_sb[:], in_=wo[jc])
                    ps_o = psum.tile([P, B], F32, tag="ps")
                    for ko in range(JC_D):
                        nc.tensor.matmul(ps_o, wo_sb[:, ko], gh_sb[:, ko],
                                         start=(ko == 0), stop=(ko == JC_D - 1))
                    nc.vector.tensor_scalar_add(o16_sb[:, jc], ps_o, bias_ap(COL_O + jc))
                    if og_sb is not o16_sb:
                        nc.vector.tensor_scalar_add(
                            og_sb[:, jc], ps_o, bias_ap(COL_O + jc)
                        )
            es_gh.close()

            es_gu = ExitStack()
            gu_pool = es_gu.enter_context(tc.tile_pool(name="gu_pool", bufs=1))
            gu_sb = gu_pool.tile([P, JC_H, B], F16)
            es_w2 = ExitStack()
            wpool2 = es_w2.enter_context(tc.tile_pool(name="wpool2", bufs=3))
            wo2_first = wpool2.tile([P, JC_H // 2, P], F16, tag="w2m")
            nc.sync.dma_start(out=wo2_first[:], in_=wo2[0, 0])

            # ---- phase 3: gu = sigmoid(t_g2)*silu(t_u)/16 -> fp16 ----
            with tc.tile_pool(name="tmp3", bufs=2) as tmp:
                for hc in range(JC_H):
                    wu_sb = wpool.tile([P, JC_D, P], F16, tag="w512")
                    nc.sync.dma_start(out=wu_sb[:], in_=wu[hc])
                    wg2_sb = wpool.tile([P, JC_D, P], OG_DT, tag="w512")
                    nc.sync.dma_start(out=wg2_sb[:], in_=wg2[hc])
                    ps_u = psum.tile([P, B], F32, tag="ps")
                    for ko in range(JC_D):
                        nc.tensor.matmul(ps_u, wu_sb[:, ko], o16_sb[:, ko],
                                         start=(ko == 0), stop=(ko == JC_D - 1))
                    ps_g2 = psum.tile([P, B], F32, tag="ps")
                    for ko in range(JC_D):
                        nc.tensor.matmul(ps_g2, wg2_sb[:, ko], og_sb[:, ko],
                                         start=(ko == 0), stop=(ko == JC_D - 1))

                    su = tmp.tile([P, B], F32, tag="su")
                    nc.scalar.activation(
                        su, ps_u, AF.Sigmoid, bias=bias_ap(COL_U + hc)
                    )
                    gg = tmp.tile([P, B], F32, tag="gg")
                    nc.scalar.activation(
                        gg, ps_g2, AF.Sigmoid, bias=bias_ap(COL_G2 + hc)
                    )
                    # u = (t_u+b_u)*sigmoid(t_u+b_u)  (f32; |u| > f16 max)
                    u_sb = tmp.tile([P, B], F32, tag="u")
                    nc.vector.scalar_tensor_tensor(
                        u_sb, ps_u, bias_ap(COL_U + hc), su, ALU.add, ALU.mult
                    )
                    # gu = (gg/16)*u -> fp16
                    nc.vector.scalar_tensor_tensor(
                        gu_sb[:, hc], gg, 1.0 / GU_SCALE, u_sb, ALU.mult, ALU.mult
                    )
            es_o.close()

            # ---- phase 4: y = proj_out(gu)*16 + b ----
            with tc.tile_pool(name="outp", bufs=2) as outp:
                for jc in range(JC_D):
                    ps_y = psum.tile([P, B], F32, tag="ps")
                    for half in range(2):
                        if jc == 0 and half == 0:
                            wo2_sb = wo2_first
                        else:
                            wo2_sb = wpool2.tile([P, JC_H // 2, P], F16, tag="w2m")
                            nc.sync.dma_start(out=wo2_sb[:], in_=wo2[jc, half])
                        for kk in range(JC_H // 2):
                            hc = half * (JC_H // 2) + kk
                            nc.tensor.matmul(
                                ps_y, wo2_sb[:, kk], gu_sb[:, hc],
                                start=(hc == 0), stop=(hc == JC_H - 1),
                            )
                    y_sb = outp.tile([P, B], F32, tag="y")
                    nc.vector.tensor_scalar(
                        y_sb, ps_y, GU_SCALE, bias_ap(COL_Y + jc),
                        ALU.mult, ALU.add,
                    )
                    nc.sync.dma_start(out=out[jc], in_=y_sb[:])
            es_w2.close()
            es_gu.close()

    _split_excess_waits(nc)
    return nc


def prep_in_maps(inputs):
    x = np.asarray(inputs["x"], np.float32)
    wo2_packed = _pack_weight(inputs["proj_out_w"])
    wo2_packed = np.ascontiguousarray(
        wo2_packed.reshape(JC_D, P, 2, JC_H // 2, P).transpose(0, 2, 1, 3, 4)
    )

    f32 = np.float32
    shared = {
        "wfr": _pack_weight(inputs["f_gate_w"], dtype=f32),
        "wcr": _pack_weight(inputs["c_proj_w"], dtype=f32),
        "wgr": _pack_weight(inputs["g_gate_w"], dtype=f32),
        "wo": _pack_weight(inputs["out_proj_w"],
                           dtype=f32 if GH_DT == F32R else np.float16),
        "wu": _pack_weight(inputs["proj_u_w"]),
        "wg2": _pack_weight(inputs["proj_g_w"],
                            dtype=f32 if OG_DT == F32R else np.float16),
        "wo2": wo2_packed,
    }
    bias = np.zeros((P, N_BIAS_COLS), np.float32)
    bias[:, COL_NF:COL_NF + JC_D] = _pack_bias_col(-np.asarray(inputs["f_gate_b"]))
    bias[:, COL_C:COL_C + JC_D] = _pack_bias_col(inputs["c_proj_b"])
    bias[:, COL_G:COL_G + JC_D] = _pack_bias_col(inputs["g_gate_b"])
    bias[:, COL_O:COL_O + JC_D] = _pack_bias_col(inputs["out_proj_b"])
    bias[:, COL_U:COL_U + JC_H] = _pack_bias_col(inputs["proj_u_b"])
    bias[:, COL_G2:COL_G2 + JC_H] = _pack_bias_col(inputs["proj_g_b"])
    bias[:, COL_Y:COL_Y + JC_D] = _pack_bias_col(inputs["proj_out_b"])
    shared["biases"] = bias

    in_maps = []
    for core in range(NCORES):
        m = dict(shared)
        xs = x[core * B : (core + 1) * B]  # [B, DIM]
        m["xr"] = np.ascontiguousarray(
            xs.reshape(B, JC_D, P).transpose(2, 1, 0)
        ).astype(np.float32)
        in_maps.append(m)
    return in_maps


def gather_output(results):
    parts = []
    for core in range(NCORES):
        y = np.asarray(results[core]["out"], np.float32)
        parts.append(y.reshape(DIM, B).T)
    return np.ascontiguousarray(np.concatenate(parts, axis=0))


_NC_CACHE = []


def run(inputs, trace=False, **kw):
    from concourse.bass_utils import run_bass_kernel_spmd

    if not _NC_CACHE:
        _NC_CACHE.append(_build_nc())
    nc = _NC_CACHE[0]
    in_maps = prep_in_maps(inputs)
    res = run_bass_kernel_spmd(
        nc, in_maps, core_ids=list(range(NCORES)), trace=trace, **kw
    )
    return res


def kernel(**inputs):
    res = run(inputs, trace=False)
    return gather_output(res.results)


# revision 3
# speedup vs baseline: 1.0115x; 1.0115x over previous
"""Trainium2 Bass kernel for nn_BitModel (MLGRU step + BitGLU, ternary weights).

Data-parallel over batch (512 rows/core, no collectives). Ternary weights
are exact in every dtype used. Stream-precision scheme (chosen from a
calibrated numpy error study; HW fp32r == round-to-nearest-12-bit-mantissa,
fp16 == 11 bits, both at ~1 matmul slot per 128-K; fp32r slots are +5%):

  x  -> f/c/g gates : fp32r single stream (12-bit)   [was fp16+fp8lo, 1.5x]
  gh -> out_proj    : fp16 (1.45e-2 total incl o_g f16; gate is 2e-2)
  o  -> proj_u/proj_g : fp16 (1.45e-2 total, under the 2e-2 gate; keeps
                        phase 3 all-f16 at 216ns slots with no dtype switches)
  gu -> proj_out    : fp16 (pre-scaled 1/16 to fit range)

All activations on ScalarE are Sigmoid (silu computed as (t+b)*sigmoid on
VectorE via scalar_tensor_tensor) so the ACT table loads once, removing
~16us of TensorE stalls the baseline had from per-chunk table reloads.
"""

import sys

sys.path.insert(0, "/opt/trn_rl_repo")

import numpy as np

import concourse.bass as bass
import concourse.mybir as mybir
import concourse.tile as tile
from concourse.vector_clock import ScopedClock

DIM = 2048
HID = 8192
BATCH = 4096
NCORES = 8
B = BATCH // NCORES
P = 128
JC_D = DIM // P   # 16
JC_H = HID // P   # 64
THRESH = 0.33
GU_SCALE = 16.0

F16 = mybir.dt.float16
F32 = mybir.dt.float32
F32R = mybir.dt.float32r

# stream dtype config (fallbacks: flip to F16 if f32r DVE-writes fail)
GH_DT = F16
OG_DT = F16

# bias column layout in the packed [128, 208] bias tensor
COL_NF = 0
COL_C = 16
COL_G = 32
COL_O = 48
COL_U = 64
COL_G2 = 128
COL_Y = 192
N_BIAS_COLS = 208

AF = mybir.ActivationFunctionType
ALU = mybir.AluOpType


def _patch_tile_drain():
    """This walrus build rejects instructions carrying >~2 attached sem
    waits; re-emit Tile's kernel-tail drain waits as standalone wait_ge."""
    if getattr(tile.TileContext, "_drain_patched", False):
        return

    def _drain_and_barrier(self, tick_clock, wait_clock):
        nc = self.nc
        probe = nc.sync.nop(nofuse=True)
        wait_clock.add_sem_waits(
            probe.ins, ScopedClock({None: tick_clock.global_clock})
        )
        si = probe.ins.sync_info
        waits = list(si.on_wait) if si else []
        if si:
            si.on_wait = []
        handles = {h.name: h for h in self.sems.allocated().values()}
        for w in waits:
            nc.sync.wait_ge(handles[w.ant_name], w.wait_value)
        nc.sync.drain()
        nc.all_engine_barrier()
        assert self.sems is not None
        popped = nc._tile_sem_poison_stack.pop()
        assert popped is self._sem_poison
        nc.clear_and_free_semaphores(list(self.sems.allocated().values()))
        nc.all_engine_barrier()

    tile.TileContext._drain_and_barrier = _drain_and_barrier
    tile.TileContext._drain_patched = True


_patch_tile_drain()


def _split_excess_waits(nc, cap=1):
    """Rewrite every instruction with >cap attached sem waits into a chain
    of single-wait InstEventSemaphore ops + the instruction with cap waits."""
    ctr = 0
    for f in nc.m.functions:
        for bb in f.blocks:
            il = bb.instructions
            i = 0
            while i < len(il):
                inst = il[i]
                si = inst.sync_info
                waits = list(si.on_wait) if si else []
                if len(waits) > cap:
                    extra, keep = waits[:-cap], waits[-cap:]
                    evs = []
                    for w in extra:
                        ev = mybir.InstEventSemaphore(
                            name=f"waitsplit-{ctr}", ins=[], outs=[]
                        )
                        ctr += 1
                        ev.engine = inst.engine
                        ev.sync_info = mybir.SyncInfo(on_wait=[w], on_update=[])
                        evs.append(ev)
                    si.on_wait = keep
                    il[i:i] = evs
                    i += len(evs)
                i += 1
    return ctr


def _ternary(w):
    w = np.asarray(w, np.float32)
    return np.where(np.abs(w) < THRESH, 0.0, np.sign(w)).astype(np.float32)


def _pack_weight(w, dtype=np.float16):
    """[out_f, in_f] f32 -> ternarized, transposed, tiled [jc, p, ko, j]
    with element = tern(w)[jc*128+j, ko*128+p]."""
    of, inf_ = w.shape
    jc, ko = of // P, inf_ // P
    t = _ternary(w).reshape(jc, P, ko, P)          # [jc, j, ko, p]
    t = np.ascontiguousarray(t.transpose(0, 3, 2, 1))  # [jc, p, ko, j]
    return t.astype(dtype)


def _pack_bias_col(b):
    return np.ascontiguousarray(np.asarray(b, np.float32).reshape(-1, P).T)


def _build_nc():
    nc = bass.Bass()

    xr = nc.declare_dram_parameter("xr", [P, JC_D, B], F32R, isOutput=False)
    wfr = nc.declare_dram_parameter("wfr", [JC_D, P, JC_D, P], F32R, isOutput=False)
    wcr = nc.declare_dram_parameter("wcr", [JC_D, P, JC_D, P], F32R, isOutput=False)
    wgr = nc.declare_dram_parameter("wgr", [JC_D, P, JC_D, P], F32R, isOutput=False)
    wo = nc.declare_dram_parameter("wo", [JC_D, P, JC_D, P], GH_DT, isOutput=False)
    wu = nc.declare_dram_parameter("wu", [JC_H, P, JC_D, P], F16, isOutput=False)
    wg2 = nc.declare_dram_parameter("wg2", [JC_H, P, JC_D, P], OG_DT, isOutput=False)
    wo2 = nc.declare_dram_parameter(
        "wo2", [JC_D, 2, P, JC_H // 2, P], F16, isOutput=False
    )
    biases = nc.declare_dram_parameter("biases", [P, N_BIAS_COLS], F32, isOutput=False)
    out = nc.declare_dram_parameter("out", [JC_D, P, B], F32, isOutput=True)

    from contextlib import ExitStack

    with tile.TileContext(nc) as tc:
        with (
            tc.tile_pool(name="const", bufs=1) as const,
            tc.tile_pool(name="wpool", bufs=4) as wpool,
            tc.tile_pool(name="psum", bufs=8, space="PSUM") as psum,
        ):
            def bias_ap(col):
                return bias_sb[:, col : col + 1]

            es_gh = ExitStack()
            gh_pool = es_gh.enter_context(tc.tile_pool(name="gh_pool", bufs=1))
            gh_sb = gh_pool.tile([P, JC_D, B], GH_DT)
            es_o = ExitStack()

            # ---- phase 1: gates; gh = g * ((1-f)*c), all-sigmoid ACT ----
            with (
                tc.tile_pool(name="x_pool", bufs=1) as x_pool,
                tc.tile_pool(name="tmp1", bufs=2) as tmp,
            ):
                # x lands as 8 separate 2-chunk tiles so the first matmuls
                # depend only on their own piece (a sliced single tile makes
                # the first matmul wait for ALL x DMAs, ~12us of dead time);
                # first-jc c/g slabs interleave into the x stream
                wf0_sb = wpool.tile([P, JC_D, P], F32R, tag="wr1m")
                nc.sync.dma_start(out=wf0_sb[:], in_=wfr[0])
                XCH = 1
                wc0_sb = wpool.tile([P, JC_D, P], F32R, tag="wr1m")
                wg0_sb = wpool.tile([P, JC_D, P], F32R, tag="wr1m")
                x_pieces = []
                for kc in range(JC_D // XCH):
                    xp = x_pool.tile([P, XCH, B], F32R, name=f"xp{kc}")
                    nc.sync.dma_start(out=xp[:], in_=xr[:, kc * XCH:(kc + 1) * XCH])
                    x_pieces.append(xp)
                    if kc == 3:
                        nc.sync.dma_start(out=wc0_sb[:], in_=wcr[0])
                    if kc == 5:
                        nc.sync.dma_start(out=wg0_sb[:], in_=wgr[0])
                bias_sb = const.tile([P, N_BIAS_COLS], F32)
                nc.sync.dma_start(out=bias_sb[:], in_=biases[:])

                def xap(ko):
                    return x_pieces[ko // XCH][:, ko % XCH]

                for jc in range(JC_D):
                    if jc == 0:
                        wf_sb = wf0_sb
                    else:
                        wf_sb = wpool.tile([P, JC_D, P], F32R, tag="wr1m")
                        nc.sync.dma_start(out=wf_sb[:], in_=wfr[jc])
                    ps_f = psum.tile([P, B], F32, tag="ps")
                    for ko in range(JC_D):
                        nc.tensor.matmul(ps_f, wf_sb[:, ko], xap(ko),
                                         start=(ko == 0), stop=(ko == JC_D - 1))

                    if jc == 0:
                        wc_sb = wc0_sb
                    else:
                        wc_sb = wpool.tile([P, JC_D, P], F32R, tag="wr1m")
                        nc.sync.dma_start(out=wc_sb[:], in_=wcr[jc])
                    ps_c = psum.tile([P, B], F32, tag="ps")
                    for ko in range(JC_D):
                        nc.tensor.matmul(ps_c, wc_sb[:, ko], xap(ko),
                                         start=(ko == 0), stop=(ko == JC_D - 1))

                    if jc == 0:
                        wg_sb = wg0_sb
                    else:
                        wg_sb = wpool.tile([P, JC_D, P], F32R, tag="wr1m")
                        nc.sync.dma_start(out=wg_sb[:], in_=wgr[jc])
                    ps_g = psum.tile([P, B], F32, tag="ps")
                    for ko in range(JC_D):
                        nc.tensor.matmul(ps_g, wg_sb[:, ko], xap(ko),
                                         start=(ko == 0), stop=(ko == JC_D - 1))

                    # 1-f = sigmoid(-(t_f+b_f)); bias col holds -b_f
                    onemf = tmp.tile([P, B], F32, tag="onemf")
                    nc.scalar.activation(
                        onemf, ps_f, AF.Sigmoid, bias=bias_ap(COL_NF + jc), scale=-1.0
                    )
                    sc = tmp.tile([P, B], F32, tag="sc")
                    nc.scalar.activation(sc, ps_c, AF.Sigmoid, bias=bias_ap(COL_C + jc))
                    g_sb = tmp.tile([P, B], F32, tag="g")
                    nc.scalar.activation(g_sb, ps_g, AF.Sigmoid, bias=bias_ap(COL_G + jc))
                    # c = silu(t_c+b_c) = (t_c+b_c)*sigmoid(t_c+b_c)
                    c_sb = tmp.tile([P, B], F32, tag="c")
                    nc.vector.scalar_tensor_tensor(
                        c_sb, ps_c, bias_ap(COL_C + jc), sc, ALU.add, ALU.mult
                    )
                    h_sb = tmp.tile([P, B], F32, tag="h")
                    nc.vector.tensor_tensor(h_sb, onemf, c_sb, ALU.mult)
                    nc.vector.tensor_tensor(gh_sb[:, jc], g_sb, h_sb, ALU.mult)

            o_pool = es_o.enter_context(
                tc.tile_pool(name="o_pool", bufs=1, side="right")
            )
            o16_sb = o_pool.tile([P, JC_D, B], F16)
            og_sb = o16_sb if OG_DT == F16 else o_pool.tile([P, JC_D, B], OG_DT)

            # ---- phase 2: o = out_proj(gh) + b -> fp16 (u) + f32r (g2) ----
            with tc.tile_pool(name="tmp2", bufs=2) as tmp:
                for jc in range(JC_D):
                    wo_sb = wpool.tile([P, JC_D, P], GH_DT, tag="wr1m")
                    nc.sync.dma_start(out=wo_sb[:], in_=wo[jc])
                    ps_o = psum.tile([P, B], F32, tag="ps")
                    for ko in range(JC_D):
                        nc.tensor.matmul(ps_o, wo_sb[:, ko], gh_sb[:, ko],
                                         start=(ko == 0), stop=(ko == JC_D - 1))
                    nc.vector.tensor_scalar_add(o16_sb[:, jc], ps_o, bias_ap(COL_O + jc))
                    if og_sb is not o16_sb:
                        nc.vector.tensor_scalar_add(
                            og_sb[:, jc], ps_o, bias_ap(COL_O + jc)
                        )
            es_gh.close()

            es_gu = ExitStack()
            gu_pool = es_gu.enter_context(tc.tile_pool(name="gu_pool", bufs=1))
            gu_sb = gu_pool.tile([P, JC_H, B], F16)
            es_w2 = ExitStack()
            wpool2 = es_w2.enter_context(tc.tile_pool(name="wpool2", bufs=3))
            wo2_first = wpool2.tile([P, JC_H // 2, P], F16, tag="w2m")
            nc.sync.dma_start(out=wo2_first[:], in_=wo2[0, 0])

            # ---- phase 3: gu = sigmoid(t_g2)*silu(t_u)/16 -> fp16 ----
            with tc.tile_pool(name="tmp3", bufs=2) as tmp:
                for hc in range(JC_H):
                    wu_sb = wpool.tile([P, JC_D, P], F16, tag="w512")
                    nc.sync.dma_start(out=wu_sb[:], in_=wu[hc])
                    wg2_sb = wpool.tile([P, JC_D, P], OG_DT, tag="w512")
                    nc.sync.dma_start(out=wg2_sb[:], in_=wg2[hc])
                    ps_u = psum.tile([P, B], F32, tag="ps")
                    for ko in range(JC_D):
                        nc.tensor.matmul(ps_u, wu_sb[:, ko], o16_sb[:, ko],
                                         start=(ko == 0), stop=(ko == JC_D - 1))
                    ps_g2 = psum.tile([P, B], F32, tag="ps")
                    for ko in range(JC_D):
                        nc.tensor.matmul(ps_g2, wg2_sb[:, ko], og_sb[:, ko],
                                         start=(ko == 0), stop=(ko == JC_D - 1))

                    su = tmp.tile([P, B], F32, tag="su")
                    nc.scalar.activation(
                        su, ps_u, AF.Sigmoid, bias=bias_ap(COL_U + hc)
                    )
                    gg = tmp.tile([P, B], F32, tag="gg")
                    nc.scalar.activation(
                        gg, ps_g2, AF.Sigmoid, bias=bias_ap(COL_G2 + hc)
                    )
                    # u = (t_u+b_u)*sigmoid(t_u+b_u)  (f32; |u| > f16 max)
                    u_sb = tmp.tile([P, B], F32, tag="u")
                    nc.vector.scalar_tensor_tensor(
                        u_sb, ps_u, bias_ap(COL_U + hc), su, ALU.add, ALU.mult
                    )
                    # gu = (gg/16)*u -> fp16
                    nc.vector.scalar_tensor_tensor(
                        gu_sb[:, hc], gg, 1.0 / GU_SCALE, u_sb, ALU.mult, ALU.mult
                    )
            es_o.close()

            # ---- phase 4: y = proj_out(gu)*16 + b ----
            with tc.tile_pool(name="outp", bufs=2) as outp:
                for jc in range(JC_D):
                    ps_y = psum.tile([P, B], F32, tag="ps")
                    for half in range(2):
                        if jc == 0 and half == 0:
                            wo2_sb = wo2_first
                        else:
                            wo2_sb = wpool2.tile([P, JC_H // 2, P], F16, tag="w2m")
                            nc.sync.dma_start(out=wo2_sb[:], in_=wo2[jc, half])
                        for kk in range(JC_H // 2):
                            hc = half * (JC_H // 2) + kk
                            nc.tensor.matmul(
                                ps_y, wo2_sb[:, kk], gu_sb[:, hc],
                                start=(hc == 0), stop=(hc == JC_H - 1),
                            )
                    y_sb = outp.tile([P, B], F32, tag="y")
                    nc.vector.tensor_scalar(
                        y_sb, ps_y, GU_SCALE, bias_ap(COL_Y + jc),
                        ALU.mult, ALU.add,
                    )
                    nc.sync.dma_start(out=out[jc], in_=y_sb[:])
            es_w2.close()
            es_gu.close()

    _split_excess_waits(nc)
    return nc


def prep_in_maps(inputs):
    x = np.asarray(inputs["x"], np.float32)
    wo2_packed = _pack_weight(inputs["proj_out_w"])
    wo2_packed = np.ascontiguousarray(
        wo2_packed.reshape(JC_D, P, 2, JC_H // 2, P).transpose(0, 2, 1, 3, 4)
    )

    f32 = np.float32
    shared = {
        "wfr": _pack_weight(inputs["f_gate_w"], dtype=f32),
        "wcr": _pack_weight(inputs["c_proj_w"], dtype=f32),
        "wgr": _pack_weight(inputs["g_gate_w"], dtype=f32),
        "wo": _pack_weight(inputs["out_proj_w"],
                           dtype=f32 if GH_DT == F32R else np.float16),
        "wu": _pack_weight(inputs["proj_u_w"]),
        "wg2": _pack_weight(inputs["proj_g_w"],
                            dtype=f32 if OG_DT == F32R else np.float16),
        "wo2": wo2_packed,
    }
    bias = np.zeros((P, N_BIAS_COLS), np.float32)
    bias[:, COL_NF:COL_NF + JC_D] = _pack_bias_col(-np.asarray(inputs["f_gate_b"]))
    bias[:, COL_C:COL_C + JC_D] = _pack_bias_col(inputs["c_proj_b"])
    bias[:, COL_G:COL_G + JC_D] = _pack_bias_col(inputs["g_gate_b"])
    bias[:, COL_O:COL_O + JC_D] = _pack_bias_col(inputs["out_proj_b"])
    bias[:, COL_U:COL_U + JC_H] = _pack_bias_col(inputs["proj_u_b"])
    bias[:, COL_G2:COL_G2 + JC_H] = _pack_bias_col(inputs["proj_g_b"])
    bias[:, COL_Y:COL_Y + JC_D] = _pack_bias_col(inputs["proj_out_b"])
    shared["biases"] = bias

    in_maps = []
    for core in range(NCORES):
        m = dict(shared)
        xs = x[core * B : (core + 1) * B]  # [B, DIM]
        m["xr"] = np.ascontiguousarray(
            xs.reshape(B, JC_D, P).transpose(2, 1, 0)
        ).astype(np.float32)
        in_maps.append(m)
    return in_maps


def gather_output(results):
    parts = []
    for core in range(NCORES):
        y = np.asarray(results[core]["out"], np.float32)
        parts.append(y.reshape(DIM, B).T)
    return np.ascontiguousarray(np.concatenate(parts, axis=0))


_NC_CACHE = []


def run(inputs, trace=False, **kw):
    from concourse.bass_utils import run_bass_kernel_spmd

    if not _NC_CACHE:
        _NC_CACHE.append(_build_nc())
    nc = _NC_CACHE[0]
    in_maps = prep_in_maps(inputs)
    res = run_bass_kernel_spmd(
        nc, in_maps, core_ids=list(range(NCORES)), trace=trace, **kw
    )
    return res


def kernel(**inputs):
    res = run(inputs, trace=False)
    return gather_output(res.results)
